# revision 2
# baseline (speedup 1.0000x reference)
"""Trainium2 Bass kernel for the Brill-Lindquist Christoffel-symbol grid.

Math: the reference reduces to
    psi  = 1 + sum_n m_n / (2 r_n),   m = softplus(pre)
    h    = psi^4
    G_c  = finite-difference gradient of h along grid axis c (2nd order
           central interior, 1st order one-sided edges, spacing DX)
    W_c  = 0.5 * G_c / h
    Gamma^i_{jk} = delta_ij W_k + delta_ik W_j - delta_jk W_i
so the [96,96,96,3,3,3] output is +-W_c scattered over 27 slots per
point (21 nonzero, 6 identically zero).

Sharding: axis 0 (12 planes per core x 8 cores). h is analytic in the
inputs, so each core evaluates its slab plus a 1-plane halo directly --
no inter-core exchange. Per core the grid is row-packed: row = a0*96+a1
(1152 rows -> 9 tiles of 128 partitions), free dim = a2 (96); h lives on
an 11-tile extended row window (halo tiles at both ends).

The h field is built once per core in fp32 (for the axis-2 shift FD and
1/h) and bf16 (matmul operand). Axis-0/1 derivatives are bf16 matmuls
against per-core FD matrices (coefficients +-0.25/DX, +-0.5/DX exact in
bf16; one-sided grid edges folded in). The h rounding to bf16 bounds the
W error by ~2^-10/DX ~ 0.05 absolute vs the ~500 the 2e-2 gate allows.

Output: device stores only the 21 nonzero slots, slot-major bf16
([row, s*96+z], 4032 B/row); the host inserts the 6 zero slots, casts
to f32 and permutes to [...,z,3,3,3]. Per-slot blocks are contiguous, so
the scatter is 6 wide ops per tile (1 for the 9 diagonal slots, 2+1 for
the +W off-diagonals, 2 for the -W) split across scalar/vector/gpsimd.
"""

import numpy as np

RES = 96
N_CORES = 8
PLANES = RES // N_CORES        # 12
LROWS = PLANES * RES           # 1152 local rows
NT = LROWS // 128              # 9 local 128-row tiles
EXTNT = NT + 2                 # 11 extended tiles (halo)
NROWS_G = RES * RES            # 9216 global rows
NSL = 21                       # stored (nonzero) output slots
OW = NSL * RES                 # 2016 free elems per output row

# bcast tile columns: crow1 | crow2 | kvec | c1 | c2
B_CR = 0
B_KV = 2 * RES
B_C = 3 * RES
BCW = 3 * RES + 2

# 27-slot -> 21-slot compression: nonzero slots of Gamma^i_{jk}, s=9i+3j+k
NZ_SLOTS = [0, 1, 2, 3, 4, 6, 8, 9, 10, 12, 13, 14, 16, 17, 18, 20, 22, 23, 24, 25, 26]

HCHUNKS = [(0, 3), (3, 6), (6, 9), (9, 11)]   # ext-block ranges for phase A


def _grid_x():
    # Match the reference grid bit-for-bit: jnp.linspace in fp32 on CPU.
    import jax
    import jax.numpy as jnp
    MAX_X = 1.0
    DX = np.float32(MAX_X / (RES / 2 - 1))

    def _ls():
        return jnp.linspace(
            DX * (1 - RES / 2), DX * (RES / 2 - 1), RES, dtype=jnp.float32
        )

    try:
        with jax.default_device(jax.devices("cpu")[0]):
            x = np.asarray(_ls())
    except Exception:
        x = np.asarray(_ls())
    return x, float(DX)


def _fd_sources(idx, coeff_c, coeff_e):
    """(offset, coeff) pairs for d/didx with 1st-order one-sided edges."""
    if idx == 0:
        return [(1, coeff_e), (0, -coeff_e)]
    if idx == RES - 1:
        return [(0, coeff_e), (-1, -coeff_e)]
    return [(1, coeff_c), (-1, -coeff_c)]


def _build_dmat(core, DX):
    """[128, 6*3*128] bf16 FD matrices as matmul lhsT ([q, p] = coeff of
    ext-row q in output row p); 0.5 Christoffel factor folded in. All
    values are +-0.25/DX or +-0.5/DX = +-11.75 / +-23.5, exact in bf16.
    Entries: 0 g0(t=0), 1 g0(interior), 2 g0(t=8), 3..5 g1(t%3)."""
    import ml_dtypes
    c0 = 0.5 * (1.0 / (2.0 * np.float64(DX)))
    ce = 0.5 * (1.0 / np.float64(DX))
    out = np.zeros((128, 6 * 3 * 128), np.float64)

    def fill(entry, t, axis):
        for p in range(128):
            gr = core * LROWS + 128 * t + p
            a = (gr // RES) if axis == 0 else (gr % RES)
            step = RES if axis == 0 else 1
            for off, cf in _fd_sources(a, c0, ce):
                g2 = gr + off * step
                e_ = g2 - core * LROWS + 128
                j = e_ // 128 - t
                q = e_ - 128 * (t + j)
                assert 0 <= j <= 2 and 0 <= q < 128, (core, t, p, off)
                out[q, (entry * 3 + j) * 128 + p] = cf

    fill(0, 0, 0)
    fill(1, 1, 0)
    fill(2, NT - 1, 0)
    for v in range(3):
        fill(3 + v, v, 1)
    return out.astype(ml_dtypes.bfloat16)


def _build_program():
    import dataclasses as _dc

    import concourse.bacc as bacc
    import concourse.mybir as mybir
    import concourse.tile as tile

    DT = mybir.dt.float32
    BF = mybir.dt.bfloat16
    AF = mybir.ActivationFunctionType

    nc = bacc.Bacc(None, target_bir_lowering=False, debug=True)
    d_dmat = nc.dram_tensor("dmat", [128, 6 * 3 * 128], BF, kind="ExternalInput")
    d_bcast = nc.dram_tensor("bcast", [128, BCW], DT, kind="ExternalInput")
    d_ab = nc.dram_tensor("ab", [128, 2 * EXTNT], DT, kind="ExternalInput")
    d_out = nc.dram_tensor("out", [LROWS, OW], BF, kind="ExternalOutput")

    HW_ = EXTNT * RES             # 1056: free width of the ext h field
    with tile.TileContext(nc) as tc:
        with (
            tc.tile_pool(name="const", bufs=1) as cpool,
            tc.tile_pool(name="chunk", bufs=2) as chpool,
            tc.tile_pool(name="work", bufs=3) as wpool,
            tc.tile_pool(name="w3", bufs=3) as w3pool,
            tc.tile_pool(name="obuf", bufs=4) as opool,
            tc.tile_pool(name="psum", bufs=4, space="PSUM") as pspool,
        ):
            # --- constants in ---
            dm = cpool.tile([128, 6 * 3 * 128], BF)
            nc.sync.dma_start(dm[:], d_dmat[:])
            B = cpool.tile([128, BCW], DT)
            nc.sync.dma_start(B[:], d_bcast[:])
            ab = cpool.tile([128, 2 * EXTNT], DT)
            nc.sync.dma_start(ab[:], d_ab[:])

            # --- phase A: h field on the extended row window ---
            H = cpool.tile([128, HW_], DT)
            Hb = cpool.tile([128, HW_], BF)
            for b0, b1 in HCHUNKS:
                nb = b1 - b0
                W = nb * RES
                csl = slice(RES * b0, RES * b1)
                qq = []
                for n in range(2):
                    crow = B[:, B_CR + RES * n:B_CR + RES * (n + 1)]
                    crow_b = _dc.replace(crow, ap=[crow.ap[0], [0, nb], [1, RES]])
                    absl = ab[:, EXTNT * n + b0:EXTNT * n + b1]
                    ab_b = _dc.replace(absl, ap=[absl.ap[0], [1, nb], [0, RES]])
                    r2 = chpool.tile([128, W], DT, tag=f"r2{n}")
                    r2v = r2[:].rearrange("p (b z) -> p b z", z=RES)
                    eng = nc.vector if n == 0 else nc.gpsimd
                    eng.tensor_add(r2v[:, :, :], crow_b, ab_b)
                    ri = chpool.tile([128, W], DT, tag=f"ri{n}")
                    nc.vector.reciprocal_approx_fast(ri[:], r2[:])
                    qn = chpool.tile([128, W], DT, tag=f"q{n}")
                    nc.scalar.activation(
                        qn[:], ri[:], AF.Sqrt, scale=B[:, B_C + n:B_C + n + 1]
                    )
                    qq.append(qn)
                ps = chpool.tile([128, W], DT, tag="ps")
                nc.gpsimd.tensor_add(ps[:], qq[0][:], qq[1][:])
                hsq = chpool.tile([128, W], DT, tag="hsq")
                nc.scalar.activation(hsq[:], ps[:], AF.Square, bias=1.0)
                nc.gpsimd.tensor_mul(H[:, csl], hsq[:], hsq[:])
                nc.scalar.activation(Hb[:, csl], hsq[:], AF.Square)

            # --- per local tile: FD matmuls, W, scatter, store ---
            for t in range(NT):
                g0e = 0 if t == 0 else (2 if t == NT - 1 else 1)
                g1e = 3 + (t % 3)
                hsl = slice(RES * (t + 1), RES * (t + 2))
                p0 = pspool.tile([128, RES], DT, tag="p0")
                p1 = pspool.tile([128, RES], DT, tag="p1")
                for ge, pp in ((g0e, p0), (g1e, p1)):
                    for j in range(3):
                        lhs = dm[:, (ge * 3 + j) * 128:(ge * 3 + j + 1) * 128]
                        rsl = slice(RES * (t + j), RES * (t + j + 1))
                        nc.tensor.matmul(
                            pp[:], lhs, Hb[:, rsl], start=(j == 0), stop=(j == 2)
                        )

                hinv = wpool.tile([128, RES], DT, tag="hinv")
                nc.vector.reciprocal_approx_fast(hinv[:], H[:, hsl])
                hz = wpool.tile([128, RES], DT, tag="hz")
                nc.gpsimd.tensor_mul(hz[:], hinv[:], B[:, B_KV:B_KV + RES])
                st = wpool.tile([128, RES], DT, tag="st")
                Ht = H[:, hsl]
                nc.gpsimd.tensor_sub(st[:, 1:95], Ht[:, 2:96], Ht[:, 0:94])
                nc.gpsimd.tensor_sub(st[:, 0:1], Ht[:, 1:2], Ht[:, 0:1])
                nc.gpsimd.tensor_sub(st[:, 95:96], Ht[:, 95:96], Ht[:, 94:95])

                w3 = w3pool.tile([128, 3 * RES], BF, tag="w3")
                nc.vector.tensor_mul(w3[:, 0:RES], p0[:], hinv[:])
                nc.vector.tensor_mul(w3[:, RES:2 * RES], p1[:], hinv[:])
                nc.vector.tensor_mul(w3[:, 2 * RES:3 * RES], st[:], hz[:])

                O = opool.tile([128, OW], BF, tag="ob")
                O3 = O[:].rearrange("p (s z) -> p s z", z=RES)
                # 9 diagonal slots: comp slots {0,1,2},{9,10,11},{18,19,20}
                # each group = [W0|W1|W2] contiguous
                ddst = _dc.replace(O[:], ap=[O[:].ap[0], [9 * RES, 3], [1, 3 * RES]])
                dsrc = _dc.replace(w3[:], ap=[w3[:].ap[0], [0, 3], [1, 3 * RES]])
                nc.scalar.copy(ddst, dsrc)
                # +W1 @ comp {3,17}; +W2 @ {5,12}; +W0 @ {8,15}
                for (s0, stp, c) in ((3, 14, 1), (5, 7, 2), (8, 7, 0)):
                    src = w3[:, c * RES:(c + 1) * RES]
                    srcb = _dc.replace(src, ap=[src.ap[0], [0, 2], [1, RES]])
                    nc.vector.tensor_copy(O3[:, s0:s0 + stp + 1:stp, :], srcb)
                # -W0 @ {4,6} and -W2 @ {14,16} merged (outer: c, inner: pair)
                nd = O[:, 4 * RES:5 * RES]
                ndst = _dc.replace(
                    nd, ap=[nd.ap[0], [10 * RES, 2], [2 * RES, 2], [1, RES]]
                )
                ns = w3[:, 0:RES]
                nsrc = _dc.replace(
                    ns, ap=[ns.ap[0], [2 * RES, 2], [0, 2], [1, RES]]
                )
                nc.gpsimd.tensor_scalar_mul(ndst, nsrc, -1.0)
                # -W1 @ {7,13}
                s1 = w3[:, RES:2 * RES]
                s1b = _dc.replace(s1, ap=[s1.ap[0], [0, 2], [1, RES]])
                nc.gpsimd.tensor_scalar_mul(O3[:, 7:14:6, :], s1b, -1.0)

                nc.sync.dma_start(d_out[128 * t:128 * (t + 1), :], O[:])

    nc.finalize()
    return nc


def _build_static():
    x, DX = _grid_x()
    dmats = [_build_dmat(c, DX) for c in range(N_CORES)]
    kvec = np.full(RES, 0.25 / DX, np.float64)
    kvec[0] = kvec[-1] = 0.5 / DX
    return x, DX, dmats, kvec.astype(np.float32)


_CACHE = {}


def _get_setup():
    if "nc" not in _CACHE:
        _CACHE["static"] = _build_static()
        _CACHE["nc"] = _build_program()
    return _CACHE["nc"], _CACHE["static"]


def _build_inmaps(BH_positions, BH_masses_presoftplus, static):
    x, DX, dmats, kvec = static
    pos = np.asarray(BH_positions, np.float64).reshape(2, 3)
    pre = np.asarray(BH_masses_presoftplus, np.float32)
    masses = np.log1p(np.exp(pre)).astype(np.float64)

    # bcast tile (identical across cores): crow1 | crow2 | kvec | c1 | c2
    bc = np.zeros((1, BCW), np.float32)
    xd = x.astype(np.float64)
    for n in range(2):
        bc[0, B_CR + RES * n:B_CR + RES * (n + 1)] = (xd - pos[n, 2]) ** 2
        bc[0, B_C + n] = (masses[n] / 2.0) ** 2
    bc[0, B_KV:B_KV + RES] = kvec
    bcast = np.ascontiguousarray(np.broadcast_to(bc, (128, BCW)))

    in_maps = []
    for c in range(N_CORES):
        slab = c * LROWS
        e = np.arange(EXTNT * 128)
        g = np.clip(slab - 128 + e, 0, NROWS_G - 1)
        xr = xd[g % RES]    # X coordinate (a1)
        yr = xd[g // RES]   # Y coordinate (a0)
        abm = np.zeros((2, EXTNT * 128), np.float64)
        for n in range(2):
            abm[n] = (xr - pos[n, 0]) ** 2 + (yr - pos[n, 1]) ** 2
        # [128, 2*EXTNT]: partition-major within each ext block
        abt = abm.reshape(2, EXTNT, 128).transpose(2, 0, 1).reshape(128, 2 * EXTNT)
        in_maps.append({
            "dmat": dmats[c],
            "bcast": bcast,
            "ab": np.ascontiguousarray(abt, np.float32),
        })
    return in_maps


def kernel(BH_positions, BH_masses_presoftplus):
    from concourse.bass_utils import run_bass_kernel_spmd

    nc, static = _get_setup()
    in_maps = _build_inmaps(BH_positions, BH_masses_presoftplus, static)
    res = run_bass_kernel_spmd(nc, in_maps, list(range(N_CORES)))

    # host gather: insert zero slots, upcast bf16 -> f32, z-major reorder
    full = np.zeros((N_CORES * LROWS, 27, RES), np.float32)
    for c in range(N_CORES):
        part = np.asarray(res.results[c]["out"]).reshape(LROWS, NSL, RES)
        full[c * LROWS:(c + 1) * LROWS, NZ_SLOTS, :] = part
    out = full.reshape(RES, RES, 27, RES).transpose(0, 1, 3, 2)
    return np.ascontiguousarray(out).reshape(RES, RES, RES, 3, 3, 3)


# revision 3
# speedup vs baseline: 2.0135x; 2.0135x over previous
"""Trainium2 Bass kernel for the Brill-Lindquist Christoffel-symbol grid.

Math: the reference reduces to
    psi  = 1 + sum_n m_n / (2 r_n),   m = softplus(pre)
    h    = psi^4
    G_c  = finite-difference gradient of h along grid axis c (2nd order
           central interior, 1st order one-sided edges, spacing DX)
    W_c  = 0.5 * G_c / h
    Gamma^i_{jk} = delta_ij W_k + delta_ik W_j - delta_jk W_i
so the [96,96,96,3,3,3] output is +-W_c scattered over 27 slots per
point (21 nonzero, 6 identically zero).

Sharding: axis 0 (12 planes per core x 8 cores). h is analytic in the
inputs, so each core evaluates its slab plus a 1-plane halo directly --
no inter-core exchange. Per core the grid is row-packed: row = a0*96+a1
(1152 rows -> 9 tiles of 128 partitions), free dim = a2 (96); h lives on
an 11-tile extended row window (halo tiles at both ends).

The h field is built once per core in fp32 (for the axis-2 shift FD and
1/h) and bf16 (matmul operand). Axis-0/1 derivatives are bf16 matmuls
against per-core FD matrices (coefficients +-0.25/DX, +-0.5/DX exact in
bf16; one-sided grid edges folded in). The h rounding to bf16 bounds the
W error by ~2^-10/DX ~ 0.05 absolute vs the ~500 the 2e-2 gate allows.

Output: device stores only the 21 nonzero slots, slot-major bf16
([row, s*96+z], 4032 B/row); the host inserts the 6 zero slots, casts
to f32 and permutes to [...,z,3,3,3]. The +-W values are cast to bf16
once per tile ([W0|W1|W2|-W0|-W1|-W2], 2 ACT-engine ops); the 21-slot
replication then runs entirely on fp32 *bitcast views* of the bf16
buffers (bf16 pairs moved as fp32 words -- every run is a multiple of
96 bf16 = 48 fp32), because DVE/Pool move fp32 at ~1 elem/cycle but
crawl on strided bf16. 6 wide copies per tile, split across engines.
"""

import numpy as np

RES = 96
N_CORES = 8
PLANES = RES // N_CORES        # 12
LROWS = PLANES * RES           # 1152 local rows
NT = LROWS // 128              # 9 local 128-row tiles
EXTNT = NT + 2                 # 11 extended tiles (halo)
NROWS_G = RES * RES            # 9216 global rows
NSL = 21                       # stored (nonzero) output slots
OW = NSL * RES                 # 2016 free elems per output row
F = RES // 2                   # 48: fp32 words per 96-bf16 slot run

# bcast tile columns: crow1 | crow2 | kvec | c1 | c2
B_CR = 0
B_KV = 2 * RES
B_C = 3 * RES
BCW = 3 * RES + 2

# 27-slot -> 21-slot compression: nonzero slots of Gamma^i_{jk}, s=9i+3j+k
NZ_SLOTS = [0, 1, 2, 3, 4, 6, 8, 9, 10, 12, 13, 14, 16, 17, 18, 20, 22, 23, 24, 25, 26]

HCHUNKS = [(0, 3), (3, 6), (6, 9), (9, 11)]   # ext-block ranges for phase A


def _grid_x():
    # Match the reference grid bit-for-bit: jnp.linspace in fp32 on CPU.
    import jax
    import jax.numpy as jnp
    MAX_X = 1.0
    DX = np.float32(MAX_X / (RES / 2 - 1))

    def _ls():
        return jnp.linspace(
            DX * (1 - RES / 2), DX * (RES / 2 - 1), RES, dtype=jnp.float32
        )

    try:
        with jax.default_device(jax.devices("cpu")[0]):
            x = np.asarray(_ls())
    except Exception:
        x = np.asarray(_ls())
    return x, float(DX)


def _fd_sources(idx, coeff_c, coeff_e):
    """(offset, coeff) pairs for d/didx with 1st-order one-sided edges."""
    if idx == 0:
        return [(1, coeff_e), (0, -coeff_e)]
    if idx == RES - 1:
        return [(0, coeff_e), (-1, -coeff_e)]
    return [(1, coeff_c), (-1, -coeff_c)]


def _build_dmat(core, DX):
    """[128, 6*3*128] bf16 FD matrices as matmul lhsT ([q, p] = coeff of
    ext-row q in output row p); 0.5 Christoffel factor folded in. All
    values are +-0.25/DX or +-0.5/DX = +-11.75 / +-23.5, exact in bf16.
    Entries: 0 g0(t=0), 1 g0(interior), 2 g0(t=8), 3..5 g1(t%3)."""
    import ml_dtypes
    c0 = 0.5 * (1.0 / (2.0 * np.float64(DX)))
    ce = 0.5 * (1.0 / np.float64(DX))
    out = np.zeros((128, 6 * 3 * 128), np.float64)

    def fill(entry, t, axis):
        for p in range(128):
            gr = core * LROWS + 128 * t + p
            a = (gr // RES) if axis == 0 else (gr % RES)
            step = RES if axis == 0 else 1
            for off, cf in _fd_sources(a, c0, ce):
                g2 = gr + off * step
                e_ = g2 - core * LROWS + 128
                j = e_ // 128 - t
                q = e_ - 128 * (t + j)
                assert 0 <= j <= 2 and 0 <= q < 128, (core, t, p, off)
                out[q, (entry * 3 + j) * 128 + p] = cf

    fill(0, 0, 0)
    fill(1, 1, 0)
    fill(2, NT - 1, 0)
    for v in range(3):
        fill(3 + v, v, 1)
    return out.astype(ml_dtypes.bfloat16)


def _build_program():
    import dataclasses as _dc

    import concourse.bacc as bacc
    import concourse.mybir as mybir
    import concourse.tile as tile
    from concourse.alu_op_type import AluOpType

    DT = mybir.dt.float32
    BF = mybir.dt.bfloat16
    AF = mybir.ActivationFunctionType

    def shift(apv, off, dims):
        return _dc.replace(apv, offset=apv.offset + off, ap=[apv.ap[0]] + dims)

    nc = bacc.Bacc(None, target_bir_lowering=False, debug=True)
    d_dmat = nc.dram_tensor("dmat", [128, 6 * 3 * 128], BF, kind="ExternalInput")
    d_bcast = nc.dram_tensor("bcast", [128, BCW], DT, kind="ExternalInput")
    d_ab = nc.dram_tensor("ab", [128, 2 * EXTNT], DT, kind="ExternalInput")
    d_out = nc.dram_tensor("out", [LROWS, OW], BF, kind="ExternalOutput")

    HW_ = EXTNT * RES             # 1056: free width of the ext h field
    with tile.TileContext(nc) as tc:
        with (
            tc.tile_pool(name="const", bufs=1) as cpool,
            tc.tile_pool(name="chunk", bufs=2) as chpool,
            tc.tile_pool(name="work", bufs=3) as wpool,
            tc.tile_pool(name="w3b", bufs=3) as w3bpool,
            tc.tile_pool(name="obuf", bufs=4) as opool,
            tc.tile_pool(name="psum", bufs=4, space="PSUM") as pspool,
        ):
            # --- constants in ---
            dm = cpool.tile([128, 6 * 3 * 128], BF)
            nc.sync.dma_start(dm[:], d_dmat[:])
            B = cpool.tile([128, BCW], DT)
            nc.sync.dma_start(B[:], d_bcast[:])
            ab = cpool.tile([128, 2 * EXTNT], DT)
            nc.sync.dma_start(ab[:], d_ab[:])

            # --- phase A: h field on the extended row window ---
            H = cpool.tile([128, HW_], DT)
            Hb = cpool.tile([128, HW_], BF)
            for b0, b1 in HCHUNKS:
                nb = b1 - b0
                W = nb * RES
                csl = slice(RES * b0, RES * b1)
                qq = []
                for n in range(2):
                    crow = B[:, B_CR + RES * n:B_CR + RES * (n + 1)]
                    crow_b = _dc.replace(crow, ap=[crow.ap[0], [0, nb], [1, RES]])
                    absl = ab[:, EXTNT * n + b0:EXTNT * n + b1]
                    ab_b = _dc.replace(absl, ap=[absl.ap[0], [1, nb], [0, RES]])
                    r2 = chpool.tile([128, W], DT, tag=f"r2{n}")
                    r2v = r2[:].rearrange("p (b z) -> p b z", z=RES)
                    eng = nc.vector if n == 0 else nc.gpsimd
                    eng.tensor_add(r2v[:, :, :], crow_b, ab_b)
                    ri = chpool.tile([128, W], DT, tag=f"ri{n}")
                    nc.vector.reciprocal_approx_fast(ri[:], r2[:])
                    qn = chpool.tile([128, W], DT, tag=f"q{n}")
                    nc.scalar.activation(
                        qn[:], ri[:], AF.Sqrt, scale=B[:, B_C + n:B_C + n + 1]
                    )
                    qq.append(qn)
                psi = chpool.tile([128, W], DT, tag="psi")
                nc.vector.scalar_tensor_tensor(
                    psi[:], qq[0][:], 1.0, qq[1][:], AluOpType.add, AluOpType.add
                )
                hsq = chpool.tile([128, W], DT, tag="hsq")
                nc.gpsimd.tensor_mul(hsq[:], psi[:], psi[:])
                nc.gpsimd.tensor_mul(H[:, csl], hsq[:], hsq[:])
                nc.scalar.activation(Hb[:, csl], hsq[:], AF.Square)

            # --- per local tile: FD matmuls, W, scatter, store ---
            for t in range(NT):
                g0e = 0 if t == 0 else (2 if t == NT - 1 else 1)
                g1e = 3 + (t % 3)
                hsl = slice(RES * (t + 1), RES * (t + 2))
                p0 = pspool.tile([128, RES], DT, tag="p0")
                p1 = pspool.tile([128, RES], DT, tag="p1")
                for ge, pp in ((g0e, p0), (g1e, p1)):
                    for j in range(3):
                        lhs = dm[:, (ge * 3 + j) * 128:(ge * 3 + j + 1) * 128]
                        rsl = slice(RES * (t + j), RES * (t + j + 1))
                        nc.tensor.matmul(
                            pp[:], lhs, Hb[:, rsl], start=(j == 0), stop=(j == 2)
                        )

                hinv = wpool.tile([128, RES], DT, tag="hinv")
                nc.vector.reciprocal_approx_fast(hinv[:], H[:, hsl])
                hz = wpool.tile([128, RES], DT, tag="hz")
                nc.gpsimd.tensor_mul(hz[:], hinv[:], B[:, B_KV:B_KV + RES])
                st = wpool.tile([128, RES], DT, tag="st")
                Ht = H[:, hsl]
                nc.gpsimd.tensor_sub(st[:, 1:95], Ht[:, 2:96], Ht[:, 0:94])
                # both one-sided edge columns in one op: st[{0,95}]
                e_d = _dc.replace(st[:], ap=[st[:].ap[0], [95, 2], [1, 1]])
                e_a = shift(Ht, 1, [[94, 2], [1, 1]])
                e_b = shift(Ht, 0, [[94, 2], [1, 1]])
                nc.gpsimd.tensor_sub(e_d, e_a, e_b)

                w3 = wpool.tile([128, 3 * RES], DT, tag="w3")
                nc.vector.tensor_mul(w3[:, 0:RES], p0[:], hinv[:])
                nc.vector.tensor_mul(w3[:, RES:2 * RES], p1[:], hinv[:])
                nc.vector.tensor_mul(w3[:, 2 * RES:3 * RES], st[:], hz[:])

                # cast to bf16 once: w3b = [W0|W1|W2|-W0|-W1|-W2] (ACT engine)
                w3b = w3bpool.tile([128, 6 * RES], BF, tag="w3b")
                nc.scalar.copy(w3b[:, 0:3 * RES], w3[:])
                nc.scalar.mul(w3b[:, 3 * RES:6 * RES], w3[:], -1.0)

                # scatter on fp32 bitcast views (all runs = 48 fp32 words)
                O = opool.tile([128, OW], BF, tag="ob")
                Ov = O[:].bitcast(DT)          # [128, 1008]
                wv = w3b[:].bitcast(DT)        # [128, 288]: 6 cols of 48
                # diag slots {0,1,2},{9,10,11},{18,19,20} <- [W0|W1|W2]
                nc.gpsimd.tensor_copy(
                    shift(Ov, 0, [[9 * F, 3], [1, 3 * F]]),
                    shift(wv, 0, [[0, 3], [1, 3 * F]]),
                )
                # +W1 @ {3,17}; +W2 @ {5,12}; +W0 @ {8,15}
                for (s0, stp, c) in ((3, 14, 1), (5, 7, 2), (8, 7, 0)):
                    nc.vector.tensor_copy(
                        shift(Ov, s0 * F, [[stp * F, 2], [1, F]]),
                        shift(wv, c * F, [[0, 2], [1, F]]),
                    )
                # -W0 @ {4,6} and -W2 @ {14,16} (outer: value col, inner: pair)
                nc.gpsimd.tensor_copy(
                    shift(Ov, 4 * F, [[10 * F, 2], [2 * F, 2], [1, F]]),
                    shift(wv, 3 * F, [[2 * F, 2], [0, 2], [1, F]]),
                )
                # -W1 @ {7,13}
                nc.scalar.copy(
                    shift(Ov, 7 * F, [[6 * F, 2], [1, F]]),
                    shift(wv, 4 * F, [[0, 2], [1, F]]),
                )

                nc.sync.dma_start(d_out[128 * t:128 * (t + 1), :], O[:])

    nc.finalize()
    return nc


def _build_static():
    x, DX = _grid_x()
    dmats = [_build_dmat(c, DX) for c in range(N_CORES)]
    kvec = np.full(RES, 0.25 / DX, np.float64)
    kvec[0] = kvec[-1] = 0.5 / DX
    return x, DX, dmats, kvec.astype(np.float32)


_CACHE = {}


def _get_setup():
    if "nc" not in _CACHE:
        _CACHE["static"] = _build_static()
        _CACHE["nc"] = _build_program()
    return _CACHE["nc"], _CACHE["static"]


def _build_inmaps(BH_positions, BH_masses_presoftplus, static):
    x, DX, dmats, kvec = static
    pos = np.asarray(BH_positions, np.float64).reshape(2, 3)
    pre = np.asarray(BH_masses_presoftplus, np.float32)
    masses = np.log1p(np.exp(pre)).astype(np.float64)

    # bcast tile (identical across cores): crow1 | crow2 | kvec | c1 | c2
    bc = np.zeros((1, BCW), np.float32)
    xd = x.astype(np.float64)
    for n in range(2):
        bc[0, B_CR + RES * n:B_CR + RES * (n + 1)] = (xd - pos[n, 2]) ** 2
        bc[0, B_C + n] = (masses[n] / 2.0) ** 2
    bc[0, B_KV:B_KV + RES] = kvec
    bcast = np.ascontiguousarray(np.broadcast_to(bc, (128, BCW)))

    in_maps = []
    for c in range(N_CORES):
        slab = c * LROWS
        e = np.arange(EXTNT * 128)
        g = np.clip(slab - 128 + e, 0, NROWS_G - 1)
        xr = xd[g % RES]    # X coordinate (a1)
        yr = xd[g // RES]   # Y coordinate (a0)
        abm = np.zeros((2, EXTNT * 128), np.float64)
        for n in range(2):
            abm[n] = (xr - pos[n, 0]) ** 2 + (yr - pos[n, 1]) ** 2
        # [128, 2*EXTNT]: partition-major within each ext block
        abt = abm.reshape(2, EXTNT, 128).transpose(2, 0, 1).reshape(128, 2 * EXTNT)
        in_maps.append({
            "dmat": dmats[c],
            "bcast": bcast,
            "ab": np.ascontiguousarray(abt, np.float32),
        })
    return in_maps


def kernel(BH_positions, BH_masses_presoftplus):
    from concourse.bass_utils import run_bass_kernel_spmd

    nc, static = _get_setup()
    in_maps = _build_inmaps(BH_positions, BH_masses_presoftplus, static)
    res = run_bass_kernel_spmd(nc, in_maps, list(range(N_CORES)))

    # host gather: insert zero slots, upcast bf16 -> f32, z-major reorder
    full = np.zeros((N_CORES * LROWS, 27, RES), np.float32)
    for c in range(N_CORES):
        part = np.asarray(res.results[c]["out"]).reshape(LROWS, NSL, RES)
        full[c * LROWS:(c + 1) * LROWS, NZ_SLOTS, :] = part
    out = full.reshape(RES, RES, 27, RES).transpose(0, 1, 3, 2)
    return np.ascontiguousarray(out).reshape(RES, RES, RES, 3, 3, 3)


# revision 8
# speedup vs baseline: 2.5592x; 1.2710x over previous
"""Trainium2 Bass kernel for the Brill-Lindquist Christoffel-symbol grid.

Math: the reference reduces to
    psi  = 1 + sum_n m_n / (2 r_n),   m = softplus(pre)
    h    = psi^4
    G_c  = finite-difference gradient of h along grid axis c (2nd order
           central interior, 1st order one-sided edges, spacing DX)
    W_c  = 0.5 * G_c / h
    Gamma^i_{jk} = delta_ij W_k + delta_ik W_j - delta_jk W_i
so the [96,96,96,3,3,3] output is +-W_c scattered over 27 slots per
point (21 nonzero, 6 identically zero).

Sharding: axis 0 (12 planes per core x 8 cores). h is analytic in the
inputs, so each core evaluates its slab plus a 1-plane halo directly --
no inter-core exchange. Per core the grid is row-packed: row = a0*96+a1
(1152 rows -> 9 tiles of 128 partitions), free dim = a2 (96); h lives on
an 11-tile extended row window (halo tiles at both ends).

The h field is built once per core in fp32 (for the axis-2 shift FD and
1/h) and bf16 (matmul operand). Axis-0/1 derivatives are bf16 matmuls
against per-core FD matrices (coefficients +-0.25/DX, +-0.5/DX exact in
bf16; one-sided grid edges folded in). The h rounding to bf16 bounds the
W error by ~2^-10/DX ~ 0.05 absolute vs the ~500 the 2e-2 gate allows.

Output: device stores only the 21 nonzero slots, slot-major bf16
([row, s*96+z], 4032 B/row); the host inserts the 6 zero slots, casts
to f32 and permutes to [...,z,3,3,3]. The compressed slot order is
value-major -- [W0 W1 W2]x5 then [-W0 -W1 -W2]x2 -- so the 21-slot
replication is done BY THE STORE DMA: per tile the +-W values are cast
to bf16 once ([W0|W1|W2|-W0|-W1|-W2], 2 ACT-engine ops) and two DMAs
with stride-0 source dims fan them out to DRAM (contiguous 2880 B +
1152 B runs per row, so no small-write HBM penalty). No SBUF scatter
ops at all -- compute engines only ever touch the 6 distinct fields.
"""

import numpy as np

RES = 96
N_CORES = 8
PLANES = RES // N_CORES        # 12
LROWS = PLANES * RES           # 1152 local rows
NT = LROWS // 128              # 9 local 128-row tiles
EXTNT = NT + 2                 # 11 extended tiles (halo)
NROWS_G = RES * RES            # 9216 global rows
NSL = 21                       # stored (nonzero) output slots
OW = NSL * RES                 # 2016 free elems per output row
F = RES // 2                   # 48: fp32 words per 96-bf16 slot run

# bcast tile columns: crow1 | crow2 | kvec | c1 | c2
B_CR = 0
B_KV = 2 * RES
B_C = 3 * RES
BCW = 3 * RES + 2

# 27-slot -> 21-slot compression, value-major device order:
# comp slot 3k+c (k<5) holds +W_c, comp slot 15+3k+c holds -W_c.
# NZ_PERM[i] = original slot (s=9i'+3j+k) whose value the i-th stored slot is.
NZ_PERM = [0, 1, 2, 10, 3, 6, 12, 13, 14, 20, 23, 16, 24, 25, 26, 4, 9, 18, 8, 17, 22]

HCHUNKS = [(0, 3), (3, 6), (6, 9), (9, 11)]   # ext-block ranges for phase A


def _grid_x():
    # Match the reference grid bit-for-bit: jnp.linspace in fp32 on CPU.
    import jax
    import jax.numpy as jnp
    MAX_X = 1.0
    DX = np.float32(MAX_X / (RES / 2 - 1))

    def _ls():
        return jnp.linspace(
            DX * (1 - RES / 2), DX * (RES / 2 - 1), RES, dtype=jnp.float32
        )

    try:
        with jax.default_device(jax.devices("cpu")[0]):
            x = np.asarray(_ls())
    except Exception:
        x = np.asarray(_ls())
    return x, float(DX)


def _fd_sources(idx, coeff_c, coeff_e):
    """(offset, coeff) pairs for d/didx with 1st-order one-sided edges."""
    if idx == 0:
        return [(1, coeff_e), (0, -coeff_e)]
    if idx == RES - 1:
        return [(0, coeff_e), (-1, -coeff_e)]
    return [(1, coeff_c), (-1, -coeff_c)]


def _build_dmat(core, DX):
    """[128, 6*3*128] bf16 FD matrices as matmul lhsT ([q, p] = coeff of
    ext-row q in output row p); 0.5 Christoffel factor folded in. All
    values are +-0.25/DX or +-0.5/DX = +-11.75 / +-23.5, exact in bf16.
    Entries: 0 g0(t=0), 1 g0(interior), 2 g0(t=8), 3..5 g1(t%3)."""
    import ml_dtypes
    c0 = 0.5 * (1.0 / (2.0 * np.float64(DX)))
    ce = 0.5 * (1.0 / np.float64(DX))
    out = np.zeros((128, 6 * 3 * 128), np.float64)

    def fill(entry, t, axis):
        for p in range(128):
            gr = core * LROWS + 128 * t + p
            a = (gr // RES) if axis == 0 else (gr % RES)
            step = RES if axis == 0 else 1
            for off, cf in _fd_sources(a, c0, ce):
                g2 = gr + off * step
                e_ = g2 - core * LROWS + 128
                j = e_ // 128 - t
                q = e_ - 128 * (t + j)
                assert 0 <= j <= 2 and 0 <= q < 128, (core, t, p, off)
                out[q, (entry * 3 + j) * 128 + p] = cf

    fill(0, 0, 0)
    fill(1, 1, 0)
    fill(2, NT - 1, 0)
    for v in range(3):
        fill(3 + v, v, 1)
    return out.astype(ml_dtypes.bfloat16)


def _build_program():
    import dataclasses as _dc

    import concourse.bacc as bacc
    import concourse.mybir as mybir
    import concourse.tile as tile
    from concourse.alu_op_type import AluOpType

    DT = mybir.dt.float32
    BF = mybir.dt.bfloat16
    AF = mybir.ActivationFunctionType

    def shift(apv, off, dims):
        return _dc.replace(apv, offset=apv.offset + off, ap=[apv.ap[0]] + dims)

    nc = bacc.Bacc(None, target_bir_lowering=False, debug=True)
    d_dmat = nc.dram_tensor("dmat", [128, 6 * 3 * 128], BF, kind="ExternalInput")
    d_bcast = nc.dram_tensor("bcast", [128, BCW], DT, kind="ExternalInput")
    d_ab = nc.dram_tensor("ab", [128, 2 * EXTNT], DT, kind="ExternalInput")
    d_out = nc.dram_tensor("out", [LROWS, OW], BF, kind="ExternalOutput")

    HW_ = EXTNT * RES             # 1056: free width of the ext h field
    with tile.TileContext(nc) as tc:
        with (
            tc.tile_pool(name="const", bufs=1) as cpool,
            tc.tile_pool(name="chunk", bufs=2) as chpool,
            tc.tile_pool(name="work", bufs=3) as wpool,
            tc.tile_pool(name="w3b", bufs=4) as w3bpool,
            tc.tile_pool(name="psum", bufs=4, space="PSUM") as pspool,
        ):
            # --- constants in ---
            dm = cpool.tile([128, 6 * 3 * 128], BF)
            nc.sync.dma_start(dm[:], d_dmat[:])
            B = cpool.tile([128, BCW], DT)
            nc.sync.dma_start(B[:], d_bcast[:])
            ab = cpool.tile([128, 2 * EXTNT], DT)
            nc.sync.dma_start(ab[:], d_ab[:])

            # --- phase A: h field on the extended row window ---
            H = cpool.tile([128, HW_], DT)
            Hb = cpool.tile([128, HW_], BF)
            for b0, b1 in HCHUNKS:
                nb = b1 - b0
                W = nb * RES
                csl = slice(RES * b0, RES * b1)
                qq = []
                for n in range(2):
                    crow = B[:, B_CR + RES * n:B_CR + RES * (n + 1)]
                    crow_b = _dc.replace(crow, ap=[crow.ap[0], [0, nb], [1, RES]])
                    absl = ab[:, EXTNT * n + b0:EXTNT * n + b1]
                    ab_b = _dc.replace(absl, ap=[absl.ap[0], [1, nb], [0, RES]])
                    r2 = chpool.tile([128, W], DT, tag=f"r2{n}")
                    r2v = r2[:].rearrange("p (b z) -> p b z", z=RES)
                    eng = nc.vector if n == 0 else nc.gpsimd
                    eng.tensor_add(r2v[:, :, :], crow_b, ab_b)
                    ri = chpool.tile([128, W], DT, tag=f"ri{n}")
                    nc.vector.reciprocal_approx_fast(ri[:], r2[:])
                    qn = chpool.tile([128, W], DT, tag=f"q{n}")
                    nc.scalar.activation(
                        qn[:], ri[:], AF.Sqrt, scale=B[:, B_C + n:B_C + n + 1]
                    )
                    qq.append(qn)
                psi = chpool.tile([128, W], DT, tag="psi")
                nc.vector.scalar_tensor_tensor(
                    psi[:], qq[0][:], 1.0, qq[1][:], AluOpType.add, AluOpType.add
                )
                hsq = chpool.tile([128, W], DT, tag="hsq")
                nc.gpsimd.tensor_mul(hsq[:], psi[:], psi[:])
                nc.gpsimd.tensor_mul(H[:, csl], hsq[:], hsq[:])
                nc.scalar.activation(Hb[:, csl], hsq[:], AF.Square)

            # --- per local tile: FD matmuls, W, scatter, store ---
            for t in range(NT):
                g0e = 0 if t == 0 else (2 if t == NT - 1 else 1)
                g1e = 3 + (t % 3)
                hsl = slice(RES * (t + 1), RES * (t + 2))
                P = pspool.tile([128, 2 * RES], DT, tag="pp")
                for half, ge in ((0, g0e), (1, g1e)):
                    for j in range(3):
                        lhs = dm[:, (ge * 3 + j) * 128:(ge * 3 + j + 1) * 128]
                        rsl = slice(RES * (t + j), RES * (t + j + 1))
                        nc.tensor.matmul(
                            P[:, RES * half:RES * (half + 1)], lhs, Hb[:, rsl],
                            start=(j == 0), stop=(j == 2)
                        )

                hinv = wpool.tile([128, RES], DT, tag="hinv")
                nc.vector.reciprocal_approx_fast(hinv[:], H[:, hsl])
                hz = wpool.tile([128, RES], DT, tag="hz")
                nc.gpsimd.tensor_mul(hz[:], hinv[:], B[:, B_KV:B_KV + RES])
                st = wpool.tile([128, RES], DT, tag="st")
                Ht = H[:, hsl]
                nc.gpsimd.tensor_sub(st[:, 1:95], Ht[:, 2:96], Ht[:, 0:94])
                # both one-sided edge columns in one op: st[{0,95}]
                e_d = _dc.replace(st[:], ap=[st[:].ap[0], [95, 2], [1, 1]])
                e_a = shift(Ht, 1, [[94, 2], [1, 1]])
                e_b = shift(Ht, 0, [[94, 2], [1, 1]])
                nc.gpsimd.tensor_sub(e_d, e_a, e_b)

                # W0|W1 in one op (hinv broadcast over the two PSUM halves)
                w3 = wpool.tile([128, 3 * RES], DT, tag="w3")
                hib = _dc.replace(
                    hinv[:], ap=[hinv[:].ap[0], [0, 2], [1, RES]]
                )
                w01 = w3[:, 0:2 * RES].rearrange("p (h z) -> p h z", z=RES)
                Pv = P[:].rearrange("p (h z) -> p h z", z=RES)
                nc.vector.tensor_mul(w01[:, :, :], Pv[:, :, :], hib)
                nc.vector.tensor_mul(w3[:, 2 * RES:3 * RES], st[:], hz[:])

                # cast to bf16 once: w3b = [W0|W1|W2|-W0|-W1|-W2] (ACT engine)
                w3b = w3bpool.tile([128, 6 * RES], BF, tag="w3b")
                nc.scalar.copy(w3b[:, 0:3 * RES], w3[:])
                nc.scalar.mul(w3b[:, 3 * RES:6 * RES], w3[:], -1.0)

                # store: the DMA replicates [W0W1W2]x5 + [-W0-W1-W2]x2 per row
                pos = w3b[:, 0:3 * RES]
                nc.sync.dma_start(
                    d_out[128 * t:128 * (t + 1), 0:15 * RES],
                    _dc.replace(pos, ap=[pos.ap[0], [0, 5], [1, 3 * RES]]),
                )
                neg = w3b[:, 3 * RES:6 * RES]
                nc.sync.dma_start(
                    d_out[128 * t:128 * (t + 1), 15 * RES:21 * RES],
                    _dc.replace(neg, ap=[neg.ap[0], [0, 2], [1, 3 * RES]]),
                )

    nc.finalize()
    return nc


def _build_static():
    x, DX = _grid_x()
    dmats = [_build_dmat(c, DX) for c in range(N_CORES)]
    kvec = np.full(RES, 0.25 / DX, np.float64)
    kvec[0] = kvec[-1] = 0.5 / DX
    return x, DX, dmats, kvec.astype(np.float32)


_CACHE = {}


def _get_setup():
    if "nc" not in _CACHE:
        _CACHE["static"] = _build_static()
        _CACHE["nc"] = _build_program()
    return _CACHE["nc"], _CACHE["static"]


def _build_inmaps(BH_positions, BH_masses_presoftplus, static):
    x, DX, dmats, kvec = static
    pos = np.asarray(BH_positions, np.float64).reshape(2, 3)
    pre = np.asarray(BH_masses_presoftplus, np.float32)
    masses = np.log1p(np.exp(pre)).astype(np.float64)

    # bcast tile (identical across cores): crow1 | crow2 | kvec | c1 | c2
    bc = np.zeros((1, BCW), np.float32)
    xd = x.astype(np.float64)
    for n in range(2):
        bc[0, B_CR + RES * n:B_CR + RES * (n + 1)] = (xd - pos[n, 2]) ** 2
        bc[0, B_C + n] = (masses[n] / 2.0) ** 2
    bc[0, B_KV:B_KV + RES] = kvec
    bcast = np.ascontiguousarray(np.broadcast_to(bc, (128, BCW)))

    in_maps = []
    for c in range(N_CORES):
        slab = c * LROWS
        e = np.arange(EXTNT * 128)
        g = np.clip(slab - 128 + e, 0, NROWS_G - 1)
        xr = xd[g % RES]    # X coordinate (a1)
        yr = xd[g // RES]   # Y coordinate (a0)
        abm = np.zeros((2, EXTNT * 128), np.float64)
        for n in range(2):
            abm[n] = (xr - pos[n, 0]) ** 2 + (yr - pos[n, 1]) ** 2
        # [128, 2*EXTNT]: partition-major within each ext block
        abt = abm.reshape(2, EXTNT, 128).transpose(2, 0, 1).reshape(128, 2 * EXTNT)
        in_maps.append({
            "dmat": dmats[c],
            "bcast": bcast,
            "ab": np.ascontiguousarray(abt, np.float32),
        })
    return in_maps


def kernel(BH_positions, BH_masses_presoftplus):
    from concourse.bass_utils import run_bass_kernel_spmd

    nc, static = _get_setup()
    in_maps = _build_inmaps(BH_positions, BH_masses_presoftplus, static)
    res = run_bass_kernel_spmd(nc, in_maps, list(range(N_CORES)))

    # host gather: insert zero slots, upcast bf16 -> f32, z-major reorder
    full = np.zeros((N_CORES * LROWS, 27, RES), np.float32)
    for c in range(N_CORES):
        part = np.asarray(res.results[c]["out"]).reshape(LROWS, NSL, RES)
        full[c * LROWS:(c + 1) * LROWS, NZ_PERM, :] = part
    out = full.reshape(RES, RES, 27, RES).transpose(0, 1, 3, 2)
    return np.ascontiguousarray(out).reshape(RES, RES, RES, 3, 3, 3)
